# revision 37
# baseline (speedup 1.0000x reference)
"""Trainium2 Bass kernel for the 21-joint hand-graph message-passing MLP.

Math (per sample b, per target joint t with neighbor list S_t of length n):
    g   = concat(x[b, S_t[0]], ..., x[b, S_t[n-1]])          # [n*64]
    h1  = relu(g @ W1_t + b1_t)                              # [128]
    h2  = relu(h1 @ W2_t + b2_t)                             # [128]
    out[b, t] = h2 @ W3_t + b3_t                             # [64]

Strategy (pure data parallel over 8 NeuronCores, B=65536 -> 8192/core):
  - x is cast fp32->bf16 on the HOST and uploaded as [BC, 21*64] bf16; the
    kernel reads it only through 11 DMA-xbar transposes per 1024-batch tile,
    producing canonical feature-major pair tiles xT[p] = [128 feats of nodes
    (2p, 2p+1), batch] (tile 10 = nodes (19, 20)).  No device-side DRAM
    round-trip (the v1 cast pre-pass was also an intermittent race).
  - L1 runs weight-stationary per target: chunks of the [n*64, 128] W1 are
    either genuine canonical node pairs (K=128), "couples" (two K=64 singles
    from opposite partition halves, co-issued back-to-back into different
    PSUM banks so the PE runs them concurrently in disjoint row groups), or
    leftover K=64 singles.
  - L2 is weight-stationary (w2 [128,128]); relu+bias fused into the PSUM
    evacuations (h1 on ScalarE activation, h2 on VectorE tensor_scalar).
  - L3 is w3-stationary (LDW=64 cols, hidden) with h2 moving at N=512: out
    lands FEATURE-major [64, batch] in PSUM; two adjacent targets share one
    PSUM bank in disjoint col groups (concurrent MMs).  b3 is a per-partition
    bias folded into the evacuation.  The feature-major result [1344, BC] is
    stored full-rate and transposed back to [BC, 21, 64] on the host during
    the unshard gather.
"""

import numpy as np
import ml_dtypes

B, J, D, H1, H2 = 65536, 21, 64, 128, 128
NCORES = 8
BC = B // NCORES          # 8192 samples per core
TILE = 1024               # batch tile (psum1/psum2 = 2 PSUM banks in fp32)
NTILES = BC // TILE       # 8
TPAIRS = 11               # L3 target pairs: (0,1),(2,3),...,(18,19),(20,)

# x pair tiles: consecutive node pairs (u, u+1) — each is one contiguous
# [1024, 128] DMA transpose.  Chosen to maximize genuine K=128 pairs in
# the L1 chunk plan (47 chunks vs 55 with the even-pair-only set).
XTILES = [(0, 1), (2, 3), (3, 4), (5, 6), (6, 7), (7, 8), (9, 10),
          (10, 11), (11, 12), (13, 14), (14, 15), (15, 16), (17, 18),
          (18, 19), (19, 20)]
NPAIRS = len(XTILES)
TILE_OF = {}              # node -> list of (tile_idx, half)
for _i, (_u, _v) in enumerate(XTILES):
    TILE_OF.setdefault(_u, []).append((_i, 0))
    TILE_OF.setdefault(_v, []).append((_i, 1))
PAIR_TILE = {(_u, _v): _i for _i, (_u, _v) in enumerate(XTILES)}

FINGER_BASE = [4 * f + 1 for f in range(5)]
NEIGH = {
    6: [[0, 1, 5, 9, 13, 17]],
    5: [[0, 5, 6, 1, 9], [0, 9, 10, 5, 13], [0, 13, 14, 9, 17]],
    4: [[0, 1, 2, 5], [0, 17, 18, 13]],
    3: [r for b in FINGER_BASE for r in ([b, b + 1, b + 2], [b + 1, b + 2, b + 3])],
    2: [[b + 2, b + 3] for b in FINGER_BASE],
}
OUT = {
    6: [0],
    5: [5, 9, 13],
    4: [1, 17],
    3: [j for b in FINGER_BASE for j in (b + 1, b + 2)],
    2: [b + 3 for b in FINGER_BASE],
}
GROUPS = [6, 5, 4, 3, 2]

# target t -> (group n, row index within its group, neighbor list)
TARGET = {}
for n in GROUPS:
    for row, t in enumerate(OUT[n]):
        TARGET[t] = (n, row, list(NEIGH[n][row]))


def build_l1_plan():
    """Per target: maximum matching of the neighbor set into XTILES pairs
    (one K=128 chunk each), remaining nodes as zero-padded singles."""
    import itertools
    plan = {}
    for t in range(21):
        n, _, S = TARGET[t]
        best = None
        # brute-force max matching (n <= 6): try pairings greedily over all
        # orders of candidate pair assignments
        cand = [(i, k) for i in range(n) for k in range(n) if i != k
                and (S[i], S[k]) in PAIR_TILE]
        def search(used, pairs, cand_left):
            nonlocal best
            if best is None or len(pairs) > len(best):
                best = list(pairs)
            for ci, (i, k) in enumerate(cand_left):
                if used[i] or used[k]:
                    continue
                used[i] = used[k] = True
                pairs.append((i, k))
                search(used, pairs, cand_left[ci + 1:])
                pairs.pop()
                used[i] = used[k] = False
        search([False] * n, [], cand)
        used = [False] * n
        pairs = []
        for i, k in best:
            pairs.append(dict(tile=PAIR_TILE[(S[i], S[k])], pos0=i, pos1=k))
            used[i] = used[k] = True
        singles = []
        for i in range(n):
            if not used[i]:
                tile_idx, half = TILE_OF[S[i]][0]
                singles.append(dict(tile=tile_idx, pos=i, half=half))
        plan[t] = dict(pairs=pairs, singles=singles)
    return plan


L1_PLAN = build_l1_plan()
# One 128-col region per chunk.  Singles are K=128 zero-padded (the other
# 64 rows are zeros): partial-row (K=64) matmuls whose row groups mix
# within a PSUM bank intermittently FAULT the device (the drain tail of
# one row group races the next row group's fill with ~ns margin), so all
# L1 matmuls use the full 128 rows and serialize safely on the PE.
def assign_w1_cols():
    cols = {}
    col = 0
    for t in range(21):
        p = L1_PLAN[t]
        for i, _ in enumerate(p["pairs"]):
            cols[(t, "pair", i)] = col
            col += 128
        for i, _ in enumerate(p["singles"]):
            cols[(t, "single", i)] = col
            col += 128
    return cols, col


W1_COLS, W1_NCOL = assign_w1_cols()


def pack_weights(inputs):
    """Host-side prep: permute/pack all weights into flat bf16/f32 arrays."""
    bf16 = ml_dtypes.bfloat16
    w1p = np.zeros((128, W1_NCOL), np.float32)
    for t in range(21):
        n, row, S = TARGET[t]
        W1 = np.asarray(inputs[f"w1_g{n}"][row], np.float32)  # [n*64, 128]
        p = L1_PLAN[t]
        for i, pr in enumerate(p["pairs"]):
            c = W1_COLS[(t, "pair", i)]
            w1p[0:64, c:c + 128] = W1[64 * pr["pos0"]:64 * pr["pos0"] + 64]
            w1p[64:128, c:c + 128] = W1[64 * pr["pos1"]:64 * pr["pos1"] + 64]
        for i, e in enumerate(p["singles"]):
            c = W1_COLS[(t, "single", i)]
            half = e["half"]
            w1p[64 * half:64 * half + 64, c:c + 128] = \
                W1[64 * e["pos"]:64 * e["pos"] + 64]
    w2p = np.zeros((128, 128 * 21), np.float32)
    w3p = np.zeros((128, 64 * 21), np.float32)
    b1p = np.zeros((128, 21), np.float32)
    b2p = np.zeros((128, 21), np.float32)
    b3p = np.zeros((128, TPAIRS), np.float32)   # per-partition bias, paired
    for t in range(21):
        n, row, _ = TARGET[t]
        w2p[:, 128 * t:128 * (t + 1)] = np.asarray(inputs[f"w2_g{n}"][row])
        w3p[:, 64 * t:64 * (t + 1)] = np.asarray(inputs[f"w3_g{n}"][row])
        b1p[:, t] = np.asarray(inputs[f"b1_g{n}"][row])
        b2p[:, t] = np.asarray(inputs[f"b2_g{n}"][row])
        b3p[64 * (t % 2):64 * (t % 2) + 64, t // 2] = \
            np.asarray(inputs[f"b3_g{n}"][row])
    return dict(
        w1p=w1p.astype(bf16), w2p=w2p.astype(bf16), w3p=w3p.astype(bf16),
        b1p=b1p, b2p=b2p, b3p=b3p,
    )


def numpy_emulate(inputs, x):
    """Bit-layout-faithful numpy model of the HW kernel (minus PSUM rounding):
    validates the chunk plan / packing / L3 pairing offline."""
    bf16 = ml_dtypes.bfloat16
    packed = pack_weights(inputs)
    xb = x.astype(bf16)  # [Bn, 21, 64]
    Bn = x.shape[0]
    xT = {}
    for p, (u, v) in enumerate(XTILES):
        xT[p] = np.concatenate([xb[:, u], xb[:, v]], 1).T
    out = np.zeros((Bn, 21, 64), np.float32)
    for t in range(21):
        psum1 = np.zeros((128, Bn), np.float32)
        pl = L1_PLAN[t]
        for i, pr in enumerate(pl["pairs"]):
            c = W1_COLS[(t, "pair", i)]
            lhsT = packed["w1p"][:, c:c + 128].astype(np.float32)
            psum1 += lhsT.T @ xT[pr["tile"]].astype(np.float32)
        for i, e in enumerate(pl["singles"]):
            c = W1_COLS[(t, "single", i)]
            lhsT = packed["w1p"][:, c:c + 128].astype(np.float32)
            psum1 += lhsT.T @ xT[e["tile"]].astype(np.float32)
        h1 = np.maximum(psum1 + packed["b1p"][:, t:t + 1], 0).astype(bf16)
        w2 = packed["w2p"][:, 128 * t:128 * (t + 1)].astype(np.float32)
        psum2 = w2.T @ h1.astype(np.float32)
        h2 = np.maximum(psum2 + packed["b2p"][:, t:t + 1], 0).astype(bf16)
        w3 = packed["w3p"][:, 64 * t:64 * (t + 1)].astype(np.float32)
        b3 = packed["b3p"][64 * (t % 2):64 * (t % 2) + 64, t // 2]
        out[:, t] = (w3.T @ h2.astype(np.float32) + b3[:, None]).T
    return out


# ---------------------------------------------------------------------------
# Bass kernel
# ---------------------------------------------------------------------------

# debug knobs: COUPLES=False issues couple singles serially (still K=64);
# L3_PAIR=False gives each target its own psum bank (no col-group pairing).
import os as _os
COUPLES = _os.environ.get("K_COUPLES", "1") == "1"
L3_PAIR = _os.environ.get("K_L3PAIR", "1") == "1"
TMAX = int(_os.environ.get("K_TMAX", "21"))      # process targets [0, TMAX)
L3OFF = _os.environ.get("K_L3OFF", "0") == "1"   # skip L3 (store psum2 raw)
TONLY = ([int(v) for v in _os.environ["K_TONLY"].split(",")]
         if "K_TONLY" in _os.environ else None)


def build_bass_kernel(num_devices=NCORES, bc=BC):
    import concourse.bass as bass
    import concourse.tile as tile
    from concourse import bacc, mybir

    bf16 = mybir.dt.bfloat16
    f32 = mybir.dt.float32
    Relu = mybir.ActivationFunctionType.Relu
    Ident = mybir.ActivationFunctionType.Identity
    Alu = mybir.AluOpType
    ntiles = bc // TILE

    nc = bacc.Bacc("TRN2", target_bir_lowering=False, debug=False,
                   num_devices=num_devices)
    x_dram = nc.dram_tensor("x", [bc, J * D], bf16, kind="ExternalInput").ap()
    out_dram = nc.dram_tensor("outf", [J * D, bc], f32, kind="ExternalOutput").ap()
    w1_dram = nc.dram_tensor("w1p", [128, W1_NCOL], bf16, kind="ExternalInput").ap()
    w2_dram = nc.dram_tensor("w2p", [128, 128 * 21], bf16, kind="ExternalInput").ap()
    w3_dram = nc.dram_tensor("w3p", [128, 64 * 21], bf16, kind="ExternalInput").ap()
    b1_dram = nc.dram_tensor("b1p", [128, 21], f32, kind="ExternalInput").ap()
    b2_dram = nc.dram_tensor("b2p", [128, 21], f32, kind="ExternalInput").ap()
    b3_dram = nc.dram_tensor("b3p", [128, TPAIRS], f32, kind="ExternalInput").ap()

    with tile.TileContext(nc) as tc:
        with (
            tc.tile_pool(name="wpool", bufs=1) as wpool,
            tc.tile_pool(name="xtp", bufs=3) as xtp,
            tc.tile_pool(name="h1p", bufs=2) as h1p,
            tc.tile_pool(name="h2p", bufs=1) as h2p,
            tc.tile_pool(name="outp", bufs=6) as outp,
            tc.tile_pool(name="ps12", bufs=3, space="PSUM") as ps12,
            tc.tile_pool(name="ps3", bufs=2, space="PSUM") as ps3,
        ):
            w1s = wpool.tile([128, W1_NCOL], bf16, name="w1s")
            w2s = wpool.tile([128, 128 * 21], bf16, name="w2s")
            w3s = wpool.tile([128, 64 * 21], bf16, name="w3s")
            b1s = wpool.tile([128, 21], f32, name="b1s")
            b2s = wpool.tile([128, 21], f32, name="b2s")
            b3s = wpool.tile([128, TPAIRS], f32, name="b3s")
            # w1 on the fast scalar HWDGE queue (needed first); the rest on
            # gpsimd so neither HWDGE queue is blocked at startup.
            nc.scalar.dma_start(w1s[:], w1_dram)
            nc.gpsimd.dma_start(w2s[:], w2_dram)
            nc.gpsimd.dma_start(w3s[:], w3_dram)
            nc.gpsimd.dma_start(b1s[:], b1_dram)
            nc.gpsimd.dma_start(b2s[:], b2_dram)
            nc.gpsimd.dma_start(b3s[:], b3_dram)

            # round-robin the L3 evacuation between the two PSUM readers;
            # ScalarE is the faster one and also carries all h1 evacs, so
            # bias the rotation toward VectorE.
            l3_rr = [0]

            # transpose issue order: target-0's tiles first so the first
            # L1 matmuls can start as early as possible
            XORDER = [0, 3, 6, 9, 12, 1, 2, 4, 5, 7, 8, 10, 11, 13, 14]

            def issue_transposes(it):
                """Feature-major pair tiles via DRAM-source xbar transpose.
                All transposes go through ONE queue (sync): the xbar unit is
                a shared resource and concurrent transposes from two queues
                corrupt each other (observed: split sync/scalar issue at it=0
                corrupted two tiles)."""
                b0 = it * TILE
                xT = [None] * NPAIRS
                for p in XORDER:
                    u = XTILES[p][0]
                    xt = xtp.tile([128, TILE], bf16, tag=f"xt{p}", name=f"xt{p}")
                    nc.sync.dma_start(
                        xt[:], x_dram[b0:b0 + TILE, 64 * u:64 * u + 128],
                        transpose=True)
                    xT[p] = xt
                return xT

            # transposes are software-pipelined one iteration ahead so the
            # sync queue issues them BEFORE iter it's store triggers (whose
            # long semaphore waits would otherwise head-of-line-block the
            # next iteration's input tiles).
            xT = issue_transposes(0)
            for it in range(ntiles):
                b0 = it * TILE
                xT_next = issue_transposes(it + 1) if it + 1 < ntiles else None

                for t in (TONLY if TONLY is not None else range(TMAX)):
                    pl = L1_PLAN[t]
                    psum1 = ps12.tile([128, TILE], f32, tag="ps12", name="ps12")
                    # all chunks are full-K (zero-padded singles): serial,
                    # race-free.  chunks = list of (w1 col, xT tile index)
                    chunks = []
                    for i, pr in enumerate(pl["pairs"]):
                        chunks.append((W1_COLS[(t, "pair", i)], pr["tile"]))
                    for i, e in enumerate(pl["singles"]):
                        chunks.append((W1_COLS[(t, "single", i)], e["tile"]))
                    nch = len(chunks)
                    for h in range(2):
                        for ci, (c, tl) in enumerate(chunks):
                            nc.tensor.matmul(
                                psum1[:, 512 * h:512 * (h + 1)],
                                w1s[:, c:c + 128],
                                xT[tl][:, 512 * h:512 * (h + 1)],
                                start=(ci == 0), stop=(ci == nch - 1))

                    h1 = h1p.tile([128, TILE], bf16, tag="h1", name="h1")
                    if t % 2 == 0:
                        nc.scalar.activation(h1[:], psum1[:], Relu,
                                             bias=b1s[:, t:t + 1], scale=1.0)
                    else:
                        nc.vector.tensor_scalar(
                            h1[:], psum1[:], b1s[:, t:t + 1], 0.0,
                            Alu.add, Alu.max)

                    psum2 = ps12.tile([128, TILE], f32, tag="ps12", name="ps12")
                    for h in range(2):
                        nc.tensor.matmul(
                            psum2[:, 512 * h:512 * (h + 1)],
                            w2s[:, 128 * t:128 * (t + 1)],
                            h1[:, 512 * h:512 * (h + 1)],
                            start=True, stop=True)
                    if L3OFF:
                        # debug: dump raw psum2 straight to the output rows
                        ot = outp.tile([128, 512], f32, tag="ot", name="ot")
                        nc.vector.tensor_copy(ot[:], psum2[:, 0:512])
                        if t < 10:
                            nc.sync.dma_start(
                                out_dram[128 * t:128 * (t + 1),
                                         b0:b0 + 512], ot[:])
                        continue
                    h2 = h2p.tile([128, TILE], bf16, tag=f"h2_{t % 4}",
                                  name=f"h2_{t % 4}")
                    if t % 2 == 1:
                        nc.scalar.activation(h2[:], psum2[:], Relu,
                                             bias=b2s[:, t:t + 1], scale=1.0)
                    else:
                        nc.vector.tensor_scalar(
                            h2[:], psum2[:], b2s[:, t:t + 1], 0.0,
                            Alu.add, Alu.max)

                    # ---- L3 for the completed target pair (t-1, t) ----
                    if (t % 2 == 1 or t == 20) and t < TMAX:
                        tp = t // 2
                        if t % 2 == 1:
                            tlo, thi = t - 1, t
                            h2lo, h2hi = h2prev, h2
                            rows = 128
                        else:
                            tlo, thi = t, None
                            h2lo, h2hi = h2, None
                            rows = 64
                        ot = outp.tile([128, TILE], f32, tag="ot", name="ot")
                        for h in range(2):
                            psum3 = ps3.tile([128, 512], f32, tag="psum3",
                                             name="psum3")
                            nc.tensor.matmul(
                                psum3[0:64, :],
                                w3s[:, 64 * tlo:64 * tlo + 64],
                                h2lo[:, 512 * h:512 * (h + 1)],
                                start=True, stop=True,
                                skip_group_check=True)
                            if thi is not None:
                                nc.tensor.matmul(
                                    psum3[64:128, :],
                                    w3s[:, 64 * thi:64 * thi + 64],
                                    h2hi[:, 512 * h:512 * (h + 1)],
                                    start=True, stop=True,
                                    skip_group_check=True)
                            l3_rr[0] = (l3_rr[0] + 1) % 11
                            if l3_rr[0] < 5:
                                nc.scalar.activation(
                                    ot[0:rows, 512 * h:512 * (h + 1)],
                                    psum3[0:rows, :], Ident,
                                    bias=b3s[0:rows, tp:tp + 1], scale=1.0)
                            else:
                                nc.vector.tensor_scalar(
                                    ot[0:rows, 512 * h:512 * (h + 1)],
                                    psum3[0:rows, :],
                                    b3s[0:rows, tp:tp + 1], None, Alu.add)
                        nc.sync.dma_start(
                            out_dram[128 * tp:128 * tp + rows,
                                     b0:b0 + TILE],
                            ot[0:rows, :])
                    h2prev = h2
                xT = xT_next

    nc.compile()
    return nc


PACKED = None
_NC = None
LAST_RESULT = None


def prepare(inputs):
    """Build (once) the bass module and the per-core input maps."""
    global PACKED, _NC
    import sys
    if "/opt/trn_rl_repo" not in sys.path:
        sys.path.insert(0, "/opt/trn_rl_repo")
    bf16 = ml_dtypes.bfloat16
    x = np.asarray(inputs["x"], np.float32).reshape(B, J * D).astype(bf16)
    PACKED = pack_weights(inputs)
    if _NC is None:
        _NC = build_bass_kernel()
    in_maps = []
    for core in range(NCORES):
        m = dict(PACKED)
        m["x"] = np.ascontiguousarray(x[core * BC:(core + 1) * BC])
        in_maps.append(m)
    return _NC, in_maps


def kernel(**inputs):
    global LAST_RESULT
    nc, in_maps = prepare(inputs)
    from concourse.bass_utils import run_bass_kernel_spmd
    res = run_bass_kernel_spmd(nc, in_maps, core_ids=list(range(NCORES)))
    LAST_RESULT = res
    # outf is [1344, BC] feature-major per core; unshard + transpose on host.
    out = np.empty((B, J, D), np.float32)
    for core, r in enumerate(res.results):
        fm = r["outf"].reshape(J, D, BC)
        out[core * BC:(core + 1) * BC] = fm.transpose(2, 0, 1)
    return out


# revision 39
# speedup vs baseline: 1.0111x; 1.0111x over previous
"""Trainium2 Bass kernel for the 21-joint hand-graph message-passing MLP.

Math (per sample b, per target joint t with neighbor list S_t of length n):
    g   = concat(x[b, S_t[0]], ..., x[b, S_t[n-1]])          # [n*64]
    h1  = relu(g @ W1_t + b1_t)                              # [128]
    h2  = relu(h1 @ W2_t + b2_t)                             # [128]
    out[b, t] = h2 @ W3_t + b3_t                             # [64]

Strategy (pure data parallel over 8 NeuronCores, B=65536 -> 8192/core):
  - x is cast fp32->bf16 on the HOST and uploaded as [BC, 21*64] bf16; the
    kernel reads it only through 11 DMA-xbar transposes per 1024-batch tile,
    producing canonical feature-major pair tiles xT[p] = [128 feats of nodes
    (2p, 2p+1), batch] (tile 10 = nodes (19, 20)).  No device-side DRAM
    round-trip (the v1 cast pre-pass was also an intermittent race).
  - L1 runs weight-stationary per target: chunks of the [n*64, 128] W1 are
    either genuine canonical node pairs (K=128), "couples" (two K=64 singles
    from opposite partition halves, co-issued back-to-back into different
    PSUM banks so the PE runs them concurrently in disjoint row groups), or
    leftover K=64 singles.
  - L2 is weight-stationary (w2 [128,128]); relu+bias fused into the PSUM
    evacuations (h1 on ScalarE activation, h2 on VectorE tensor_scalar).
  - L3 is w3-stationary (LDW=64 cols, hidden) with h2 moving at N=512: out
    lands FEATURE-major [64, batch] in PSUM; two adjacent targets share one
    PSUM bank in disjoint col groups (concurrent MMs).  b3 is a per-partition
    bias folded into the evacuation.  The feature-major result [1344, BC] is
    stored full-rate and transposed back to [BC, 21, 64] on the host during
    the unshard gather.
"""

import numpy as np
import ml_dtypes

B, J, D, H1, H2 = 65536, 21, 64, 128, 128
NCORES = 8
BC = B // NCORES          # 8192 samples per core
TILE = 1024               # batch tile (psum1/psum2 = 2 PSUM banks in fp32)
NTILES = BC // TILE       # 8
TPAIRS = 11               # L3 target pairs: (0,1),(2,3),...,(18,19),(20,)

# x pair tiles: consecutive node pairs (u, u+1) — each is one contiguous
# [1024, 128] DMA transpose.  Chosen to maximize genuine K=128 pairs in
# the L1 chunk plan (47 chunks vs 55 with the even-pair-only set).
XTILES = [(0, 1), (2, 3), (3, 4), (5, 6), (6, 7), (7, 8), (9, 10),
          (10, 11), (11, 12), (13, 14), (14, 15), (15, 16), (17, 18),
          (18, 19), (19, 20)]
NPAIRS = len(XTILES)
TILE_OF = {}              # node -> list of (tile_idx, half)
for _i, (_u, _v) in enumerate(XTILES):
    TILE_OF.setdefault(_u, []).append((_i, 0))
    TILE_OF.setdefault(_v, []).append((_i, 1))
PAIR_TILE = {(_u, _v): _i for _i, (_u, _v) in enumerate(XTILES)}

FINGER_BASE = [4 * f + 1 for f in range(5)]
NEIGH = {
    6: [[0, 1, 5, 9, 13, 17]],
    5: [[0, 5, 6, 1, 9], [0, 9, 10, 5, 13], [0, 13, 14, 9, 17]],
    4: [[0, 1, 2, 5], [0, 17, 18, 13]],
    3: [r for b in FINGER_BASE for r in ([b, b + 1, b + 2], [b + 1, b + 2, b + 3])],
    2: [[b + 2, b + 3] for b in FINGER_BASE],
}
OUT = {
    6: [0],
    5: [5, 9, 13],
    4: [1, 17],
    3: [j for b in FINGER_BASE for j in (b + 1, b + 2)],
    2: [b + 3 for b in FINGER_BASE],
}
GROUPS = [6, 5, 4, 3, 2]

# target t -> (group n, row index within its group, neighbor list)
TARGET = {}
for n in GROUPS:
    for row, t in enumerate(OUT[n]):
        TARGET[t] = (n, row, list(NEIGH[n][row]))


def build_l1_plan():
    """Per target: maximum matching of the neighbor set into XTILES pairs
    (one K=128 chunk each), remaining nodes as zero-padded singles."""
    import itertools
    plan = {}
    for t in range(21):
        n, _, S = TARGET[t]
        best = None
        # brute-force max matching (n <= 6): try pairings greedily over all
        # orders of candidate pair assignments
        cand = [(i, k) for i in range(n) for k in range(n) if i != k
                and (S[i], S[k]) in PAIR_TILE]
        def search(used, pairs, cand_left):
            nonlocal best
            if best is None or len(pairs) > len(best):
                best = list(pairs)
            for ci, (i, k) in enumerate(cand_left):
                if used[i] or used[k]:
                    continue
                used[i] = used[k] = True
                pairs.append((i, k))
                search(used, pairs, cand_left[ci + 1:])
                pairs.pop()
                used[i] = used[k] = False
        search([False] * n, [], cand)
        used = [False] * n
        pairs = []
        for i, k in best:
            pairs.append(dict(tile=PAIR_TILE[(S[i], S[k])], pos0=i, pos1=k))
            used[i] = used[k] = True
        singles = []
        for i in range(n):
            if not used[i]:
                tile_idx, half = TILE_OF[S[i]][0]
                singles.append(dict(tile=tile_idx, pos=i, half=half))
        plan[t] = dict(pairs=pairs, singles=singles)
    return plan


L1_PLAN = build_l1_plan()
# One 128-col region per chunk.  Singles are K=128 zero-padded (the other
# 64 rows are zeros): partial-row (K=64) matmuls whose row groups mix
# within a PSUM bank intermittently FAULT the device (the drain tail of
# one row group races the next row group's fill with ~ns margin), so all
# L1 matmuls use the full 128 rows and serialize safely on the PE.
def assign_w1_cols():
    cols = {}
    col = 0
    for t in range(21):
        p = L1_PLAN[t]
        for i, _ in enumerate(p["pairs"]):
            cols[(t, "pair", i)] = col
            col += 128
        for i, _ in enumerate(p["singles"]):
            cols[(t, "single", i)] = col
            col += 128
    return cols, col


W1_COLS, W1_NCOL = assign_w1_cols()


def pack_weights(inputs):
    """Host-side prep: permute/pack all weights into flat bf16/f32 arrays."""
    bf16 = ml_dtypes.bfloat16
    w1p = np.zeros((128, W1_NCOL), np.float32)
    for t in range(21):
        n, row, S = TARGET[t]
        W1 = np.asarray(inputs[f"w1_g{n}"][row], np.float32)  # [n*64, 128]
        p = L1_PLAN[t]
        for i, pr in enumerate(p["pairs"]):
            c = W1_COLS[(t, "pair", i)]
            w1p[0:64, c:c + 128] = W1[64 * pr["pos0"]:64 * pr["pos0"] + 64]
            w1p[64:128, c:c + 128] = W1[64 * pr["pos1"]:64 * pr["pos1"] + 64]
        for i, e in enumerate(p["singles"]):
            c = W1_COLS[(t, "single", i)]
            half = e["half"]
            w1p[64 * half:64 * half + 64, c:c + 128] = \
                W1[64 * e["pos"]:64 * e["pos"] + 64]
    w2p = np.zeros((128, 128 * 21), np.float32)
    w3p = np.zeros((128, 64 * 21), np.float32)
    b1p = np.zeros((128, 21), np.float32)
    b2p = np.zeros((128, 21), np.float32)
    b3p = np.zeros((128, TPAIRS), np.float32)   # per-partition bias, paired
    for t in range(21):
        n, row, _ = TARGET[t]
        w2p[:, 128 * t:128 * (t + 1)] = np.asarray(inputs[f"w2_g{n}"][row])
        w3p[:, 64 * t:64 * (t + 1)] = np.asarray(inputs[f"w3_g{n}"][row])
        b1p[:, t] = np.asarray(inputs[f"b1_g{n}"][row])
        b2p[:, t] = np.asarray(inputs[f"b2_g{n}"][row])
        b3p[64 * (t % 2):64 * (t % 2) + 64, t // 2] = \
            np.asarray(inputs[f"b3_g{n}"][row])
    return dict(
        w1p=w1p.astype(bf16), w2p=w2p.astype(bf16), w3p=w3p.astype(bf16),
        b1p=b1p, b2p=b2p, b3p=b3p,
    )


def numpy_emulate(inputs, x):
    """Bit-layout-faithful numpy model of the HW kernel (minus PSUM rounding):
    validates the chunk plan / packing / L3 pairing offline."""
    bf16 = ml_dtypes.bfloat16
    packed = pack_weights(inputs)
    xb = x.astype(bf16)  # [Bn, 21, 64]
    Bn = x.shape[0]
    xT = {}
    for p, (u, v) in enumerate(XTILES):
        xT[p] = np.concatenate([xb[:, u], xb[:, v]], 1).T
    out = np.zeros((Bn, 21, 64), np.float32)
    for t in range(21):
        psum1 = np.zeros((128, Bn), np.float32)
        pl = L1_PLAN[t]
        for i, pr in enumerate(pl["pairs"]):
            c = W1_COLS[(t, "pair", i)]
            lhsT = packed["w1p"][:, c:c + 128].astype(np.float32)
            psum1 += lhsT.T @ xT[pr["tile"]].astype(np.float32)
        for i, e in enumerate(pl["singles"]):
            c = W1_COLS[(t, "single", i)]
            lhsT = packed["w1p"][:, c:c + 128].astype(np.float32)
            psum1 += lhsT.T @ xT[e["tile"]].astype(np.float32)
        h1 = np.maximum(psum1 + packed["b1p"][:, t:t + 1], 0).astype(bf16)
        w2 = packed["w2p"][:, 128 * t:128 * (t + 1)].astype(np.float32)
        psum2 = w2.T @ h1.astype(np.float32)
        h2 = np.maximum(psum2 + packed["b2p"][:, t:t + 1], 0).astype(bf16)
        w3 = packed["w3p"][:, 64 * t:64 * (t + 1)].astype(np.float32)
        b3 = packed["b3p"][64 * (t % 2):64 * (t % 2) + 64, t // 2]
        out[:, t] = (w3.T @ h2.astype(np.float32) + b3[:, None]).T
    return out


# ---------------------------------------------------------------------------
# Bass kernel
# ---------------------------------------------------------------------------

# debug knobs: COUPLES=False issues couple singles serially (still K=64);
# L3_PAIR=False gives each target its own psum bank (no col-group pairing).
import os as _os
COUPLES = _os.environ.get("K_COUPLES", "1") == "1"
L3_PAIR = _os.environ.get("K_L3PAIR", "1") == "1"
TMAX = int(_os.environ.get("K_TMAX", "21"))      # process targets [0, TMAX)
L3OFF = _os.environ.get("K_L3OFF", "0") == "1"   # skip L3 (store psum2 raw)
TONLY = ([int(v) for v in _os.environ["K_TONLY"].split(",")]
         if "K_TONLY" in _os.environ else None)


def build_bass_kernel(num_devices=NCORES, bc=BC):
    import concourse.bass as bass
    import concourse.tile as tile
    from concourse import bacc, mybir

    bf16 = mybir.dt.bfloat16
    f32 = mybir.dt.float32
    Relu = mybir.ActivationFunctionType.Relu
    Ident = mybir.ActivationFunctionType.Identity
    Alu = mybir.AluOpType
    ntiles = bc // TILE

    nc = bacc.Bacc("TRN2", target_bir_lowering=False, debug=False,
                   num_devices=num_devices)
    x_dram = nc.dram_tensor("x", [bc, J * D], bf16, kind="ExternalInput").ap()
    out_dram = nc.dram_tensor("outf", [J * D, bc], f32, kind="ExternalOutput").ap()
    w1_dram = nc.dram_tensor("w1p", [128, W1_NCOL], bf16, kind="ExternalInput").ap()
    w2_dram = nc.dram_tensor("w2p", [128, 128 * 21], bf16, kind="ExternalInput").ap()
    w3_dram = nc.dram_tensor("w3p", [128, 64 * 21], bf16, kind="ExternalInput").ap()
    b1_dram = nc.dram_tensor("b1p", [128, 21], f32, kind="ExternalInput").ap()
    b2_dram = nc.dram_tensor("b2p", [128, 21], f32, kind="ExternalInput").ap()
    b3_dram = nc.dram_tensor("b3p", [128, TPAIRS], f32, kind="ExternalInput").ap()

    with tile.TileContext(nc) as tc:
        with (
            tc.tile_pool(name="wpool", bufs=1) as wpool,
            tc.tile_pool(name="xtp", bufs=3) as xtp,
            tc.tile_pool(name="h1p", bufs=2) as h1p,
            tc.tile_pool(name="h2p", bufs=1) as h2p,
            tc.tile_pool(name="outp", bufs=6) as outp,
            tc.tile_pool(name="ps12", bufs=3, space="PSUM") as ps12,
            tc.tile_pool(name="ps3", bufs=2, space="PSUM") as ps3,
        ):
            w1s = wpool.tile([128, W1_NCOL], bf16, name="w1s")
            w2s = wpool.tile([128, 128 * 21], bf16, name="w2s")
            w3s = wpool.tile([128, 64 * 21], bf16, name="w3s")
            b1s = wpool.tile([128, 21], f32, name="b1s")
            b2s = wpool.tile([128, 21], f32, name="b2s")
            b3s = wpool.tile([128, TPAIRS], f32, name="b3s")
            # w1 on the fast scalar HWDGE queue (needed first); the rest on
            # gpsimd so neither HWDGE queue is blocked at startup.
            nc.scalar.dma_start(w1s[:], w1_dram)
            nc.gpsimd.dma_start(w2s[:], w2_dram)
            nc.gpsimd.dma_start(w3s[:], w3_dram)
            nc.gpsimd.dma_start(b1s[:], b1_dram)
            nc.gpsimd.dma_start(b2s[:], b2_dram)
            nc.gpsimd.dma_start(b3s[:], b3_dram)

            # round-robin the L3 evacuation between the two PSUM readers;
            # ScalarE is the faster one and also carries all h1 evacs, so
            # bias the rotation toward VectorE.
            l3_rr = [0]

            # transpose issue order: target-0's tiles first so the first
            # L1 matmuls can start as early as possible
            XORDER = [0, 3, 6, 9, 12, 1, 2, 4, 5, 7, 8, 10, 11, 13, 14]

            def issue_transposes(it):
                """Feature-major pair tiles via DRAM-source xbar transpose.
                All transposes go through ONE queue (sync): the xbar unit is
                a shared resource and concurrent transposes from two queues
                corrupt each other (observed: split sync/scalar issue at it=0
                corrupted two tiles)."""
                b0 = it * TILE
                xT = [None] * NPAIRS
                for p in XORDER:
                    u = XTILES[p][0]
                    xt = xtp.tile([128, TILE], bf16, tag=f"xt{p}", name=f"xt{p}")
                    nc.sync.dma_start(
                        xt[:], x_dram[b0:b0 + TILE, 64 * u:64 * u + 128],
                        transpose=True)
                    xT[p] = xt
                return xT

            # transposes are software-pipelined one iteration ahead so the
            # sync queue issues them BEFORE iter it's store triggers (whose
            # long semaphore waits would otherwise head-of-line-block the
            # next iteration's input tiles).
            xT = issue_transposes(0)
            for it in range(ntiles):
                b0 = it * TILE
                xT_next = issue_transposes(it + 1) if it + 1 < ntiles else None

                for t in (TONLY if TONLY is not None else range(TMAX)):
                    pl = L1_PLAN[t]
                    psum1 = ps12.tile([128, TILE], f32, tag="ps12", name="ps12")
                    # all chunks are full-K (zero-padded singles): serial,
                    # race-free.  chunks = list of (w1 col, xT tile index)
                    chunks = []
                    for i, pr in enumerate(pl["pairs"]):
                        chunks.append((W1_COLS[(t, "pair", i)], pr["tile"]))
                    for i, e in enumerate(pl["singles"]):
                        chunks.append((W1_COLS[(t, "single", i)], e["tile"]))
                    nch = len(chunks)
                    # h innermost: consecutive matmuls share the stationary
                    # operand, letting codegen skip redundant weight loads
                    for ci, (c, tl) in enumerate(chunks):
                        for h in range(2):
                            nc.tensor.matmul(
                                psum1[:, 512 * h:512 * (h + 1)],
                                w1s[:, c:c + 128],
                                xT[tl][:, 512 * h:512 * (h + 1)],
                                start=(ci == 0), stop=(ci == nch - 1))

                    h1 = h1p.tile([128, TILE], bf16, tag="h1", name="h1")
                    if t % 2 == 0:
                        nc.scalar.activation(h1[:], psum1[:], Relu,
                                             bias=b1s[:, t:t + 1], scale=1.0)
                    else:
                        nc.vector.tensor_scalar(
                            h1[:], psum1[:], b1s[:, t:t + 1], 0.0,
                            Alu.add, Alu.max)

                    psum2 = ps12.tile([128, TILE], f32, tag="ps12", name="ps12")
                    for h in range(2):
                        nc.tensor.matmul(
                            psum2[:, 512 * h:512 * (h + 1)],
                            w2s[:, 128 * t:128 * (t + 1)],
                            h1[:, 512 * h:512 * (h + 1)],
                            start=True, stop=True)
                    if L3OFF:
                        # debug: dump raw psum2 straight to the output rows
                        ot = outp.tile([128, 512], f32, tag="ot", name="ot")
                        nc.vector.tensor_copy(ot[:], psum2[:, 0:512])
                        if t < 10:
                            nc.sync.dma_start(
                                out_dram[128 * t:128 * (t + 1),
                                         b0:b0 + 512], ot[:])
                        continue
                    h2 = h2p.tile([128, TILE], bf16, tag=f"h2_{t % 4}",
                                  name=f"h2_{t % 4}")
                    if t % 2 == 1:
                        nc.scalar.activation(h2[:], psum2[:], Relu,
                                             bias=b2s[:, t:t + 1], scale=1.0)
                    else:
                        nc.vector.tensor_scalar(
                            h2[:], psum2[:], b2s[:, t:t + 1], 0.0,
                            Alu.add, Alu.max)

                    # ---- L3 for the completed target pair (t-1, t) ----
                    if (t % 2 == 1 or t == 20) and t < TMAX:
                        tp = t // 2
                        if t % 2 == 1:
                            tlo, thi = t - 1, t
                            h2lo, h2hi = h2prev, h2
                            rows = 128
                        else:
                            tlo, thi = t, None
                            h2lo, h2hi = h2, None
                            rows = 64
                        ot = outp.tile([128, TILE], f32, tag="ot", name="ot")
                        ps3a = ps3.tile([128, 512], f32, tag="psum3",
                                        name="psum3")
                        ps3b = ps3.tile([128, 512], f32, tag="psum3",
                                        name="psum3")
                        # h innermost per stationary (w3_t) so consecutive
                        # matmuls can reuse the loaded weights
                        for h in range(2):
                            nc.tensor.matmul(
                                (ps3a if h == 0 else ps3b)[0:64, :],
                                w3s[:, 64 * tlo:64 * tlo + 64],
                                h2lo[:, 512 * h:512 * (h + 1)],
                                start=True, stop=True,
                                skip_group_check=True)
                        if thi is not None:
                            for h in range(2):
                                nc.tensor.matmul(
                                    (ps3a if h == 0 else ps3b)[64:128, :],
                                    w3s[:, 64 * thi:64 * thi + 64],
                                    h2hi[:, 512 * h:512 * (h + 1)],
                                    start=True, stop=True,
                                    skip_group_check=True)
                        for h in range(2):
                            psum3 = ps3a if h == 0 else ps3b
                            l3_rr[0] = (l3_rr[0] + 1) % 11
                            if l3_rr[0] < 5:
                                nc.scalar.activation(
                                    ot[0:rows, 512 * h:512 * (h + 1)],
                                    psum3[0:rows, :], Ident,
                                    bias=b3s[0:rows, tp:tp + 1], scale=1.0)
                            else:
                                nc.vector.tensor_scalar(
                                    ot[0:rows, 512 * h:512 * (h + 1)],
                                    psum3[0:rows, :],
                                    b3s[0:rows, tp:tp + 1], None, Alu.add)
                        nc.sync.dma_start(
                            out_dram[128 * tp:128 * tp + rows,
                                     b0:b0 + TILE],
                            ot[0:rows, :])
                    h2prev = h2
                xT = xT_next

    nc.compile()
    return nc


PACKED = None
_NC = None
LAST_RESULT = None


def prepare(inputs):
    """Build (once) the bass module and the per-core input maps."""
    global PACKED, _NC
    import sys
    if "/opt/trn_rl_repo" not in sys.path:
        sys.path.insert(0, "/opt/trn_rl_repo")
    bf16 = ml_dtypes.bfloat16
    x = np.asarray(inputs["x"], np.float32).reshape(B, J * D).astype(bf16)
    PACKED = pack_weights(inputs)
    if _NC is None:
        _NC = build_bass_kernel()
    in_maps = []
    for core in range(NCORES):
        m = dict(PACKED)
        m["x"] = np.ascontiguousarray(x[core * BC:(core + 1) * BC])
        in_maps.append(m)
    return _NC, in_maps


def kernel(**inputs):
    global LAST_RESULT
    nc, in_maps = prepare(inputs)
    from concourse.bass_utils import run_bass_kernel_spmd
    res = run_bass_kernel_spmd(nc, in_maps, core_ids=list(range(NCORES)))
    LAST_RESULT = res
    # outf is [1344, BC] feature-major per core; unshard + transpose on host.
    out = np.empty((B, J, D), np.float32)
    for core, r in enumerate(res.results):
        fm = r["outf"].reshape(J, D, BC)
        out[core * BC:(core + 1) * BC] = fm.transpose(2, 0, 1)
    return out


# revision 40
# speedup vs baseline: 1.1013x; 1.0893x over previous
"""Trainium2 Bass kernel for the 21-joint hand-graph message-passing MLP.

Math (per sample b, per target joint t with neighbor list S_t of length n):
    g   = concat(x[b, S_t[0]], ..., x[b, S_t[n-1]])          # [n*64]
    h1  = relu(g @ W1_t + b1_t)                              # [128]
    h2  = relu(h1 @ W2_t + b2_t)                             # [128]
    out[b, t] = h2 @ W3_t + b3_t                             # [64]

Strategy (pure data parallel over 8 NeuronCores, B=65536 -> 8192/core):
  - x is cast fp32->bf16 on the HOST and uploaded as [BC, 21*64] bf16; the
    kernel reads it only through 11 DMA-xbar transposes per 1024-batch tile,
    producing canonical feature-major pair tiles xT[p] = [128 feats of nodes
    (2p, 2p+1), batch] (tile 10 = nodes (19, 20)).  No device-side DRAM
    round-trip (the v1 cast pre-pass was also an intermittent race).
  - L1 runs weight-stationary per target: chunks of the [n*64, 128] W1 are
    either genuine canonical node pairs (K=128), "couples" (two K=64 singles
    from opposite partition halves, co-issued back-to-back into different
    PSUM banks so the PE runs them concurrently in disjoint row groups), or
    leftover K=64 singles.
  - L2 is weight-stationary (w2 [128,128]); relu+bias fused into the PSUM
    evacuations (h1 on ScalarE activation, h2 on VectorE tensor_scalar).
  - L3 is w3-stationary (LDW=64 cols, hidden) with h2 moving at N=512: out
    lands FEATURE-major [64, batch] in PSUM; two adjacent targets share one
    PSUM bank in disjoint col groups (concurrent MMs).  b3 is a per-partition
    bias folded into the evacuation.  The feature-major result [1344, BC] is
    stored full-rate and transposed back to [BC, 21, 64] on the host during
    the unshard gather.
"""

import numpy as np
import ml_dtypes

B, J, D, H1, H2 = 65536, 21, 64, 128, 128
NCORES = 8
BC = B // NCORES          # 8192 samples per core
TILE = 1024               # batch tile (psum1/psum2 = 2 PSUM banks in fp32)
NTILES = BC // TILE       # 8
TPAIRS = 11               # L3 target pairs: (0,1),(2,3),...,(18,19),(20,)

# x pair tiles: consecutive node pairs (u, u+1) — each is one contiguous
# [1024, 128] DMA transpose.  Chosen to maximize genuine K=128 pairs in
# the L1 chunk plan (47 chunks vs 55 with the even-pair-only set).
XTILES = [(0, 1), (2, 3), (3, 4), (5, 6), (6, 7), (7, 8), (9, 10),
          (10, 11), (11, 12), (13, 14), (14, 15), (15, 16), (17, 18),
          (18, 19), (19, 20)]
NPAIRS = len(XTILES)
TILE_OF = {}              # node -> list of (tile_idx, half)
for _i, (_u, _v) in enumerate(XTILES):
    TILE_OF.setdefault(_u, []).append((_i, 0))
    TILE_OF.setdefault(_v, []).append((_i, 1))
PAIR_TILE = {(_u, _v): _i for _i, (_u, _v) in enumerate(XTILES)}

FINGER_BASE = [4 * f + 1 for f in range(5)]
NEIGH = {
    6: [[0, 1, 5, 9, 13, 17]],
    5: [[0, 5, 6, 1, 9], [0, 9, 10, 5, 13], [0, 13, 14, 9, 17]],
    4: [[0, 1, 2, 5], [0, 17, 18, 13]],
    3: [r for b in FINGER_BASE for r in ([b, b + 1, b + 2], [b + 1, b + 2, b + 3])],
    2: [[b + 2, b + 3] for b in FINGER_BASE],
}
OUT = {
    6: [0],
    5: [5, 9, 13],
    4: [1, 17],
    3: [j for b in FINGER_BASE for j in (b + 1, b + 2)],
    2: [b + 3 for b in FINGER_BASE],
}
GROUPS = [6, 5, 4, 3, 2]

# target t -> (group n, row index within its group, neighbor list)
TARGET = {}
for n in GROUPS:
    for row, t in enumerate(OUT[n]):
        TARGET[t] = (n, row, list(NEIGH[n][row]))


def build_l1_plan():
    """Per target: maximum matching of the neighbor set into XTILES pairs
    (one K=128 chunk each), remaining nodes as zero-padded singles."""
    import itertools
    plan = {}
    for t in range(21):
        n, _, S = TARGET[t]
        best = None
        # brute-force max matching (n <= 6): try pairings greedily over all
        # orders of candidate pair assignments
        cand = [(i, k) for i in range(n) for k in range(n) if i != k
                and (S[i], S[k]) in PAIR_TILE]
        def search(used, pairs, cand_left):
            nonlocal best
            if best is None or len(pairs) > len(best):
                best = list(pairs)
            for ci, (i, k) in enumerate(cand_left):
                if used[i] or used[k]:
                    continue
                used[i] = used[k] = True
                pairs.append((i, k))
                search(used, pairs, cand_left[ci + 1:])
                pairs.pop()
                used[i] = used[k] = False
        search([False] * n, [], cand)
        used = [False] * n
        pairs = []
        for i, k in best:
            pairs.append(dict(tile=PAIR_TILE[(S[i], S[k])], pos0=i, pos1=k))
            used[i] = used[k] = True
        singles = []
        for i in range(n):
            if not used[i]:
                tile_idx, half = TILE_OF[S[i]][0]
                singles.append(dict(tile=tile_idx, pos=i, half=half))
        plan[t] = dict(pairs=pairs, singles=singles)
    return plan


L1_PLAN = build_l1_plan()
# One 128-col region per chunk.  Singles are K=128 zero-padded (the other
# 64 rows are zeros): partial-row (K=64) matmuls whose row groups mix
# within a PSUM bank intermittently FAULT the device (the drain tail of
# one row group races the next row group's fill with ~ns margin), so all
# L1 matmuls use the full 128 rows and serialize safely on the PE.
def assign_w1_cols():
    cols = {}
    col = 0
    for t in range(21):
        p = L1_PLAN[t]
        for i, _ in enumerate(p["pairs"]):
            cols[(t, "pair", i)] = col
            col += 128
        for i, _ in enumerate(p["singles"]):
            cols[(t, "single", i)] = col
            col += 128
    return cols, col


W1_COLS, W1_NCOL = assign_w1_cols()


def pack_weights(inputs):
    """Host-side prep: permute/pack all weights into flat bf16/f32 arrays."""
    bf16 = ml_dtypes.bfloat16
    w1p = np.zeros((128, W1_NCOL), np.float32)
    for t in range(21):
        n, row, S = TARGET[t]
        W1 = np.asarray(inputs[f"w1_g{n}"][row], np.float32)  # [n*64, 128]
        p = L1_PLAN[t]
        for i, pr in enumerate(p["pairs"]):
            c = W1_COLS[(t, "pair", i)]
            w1p[0:64, c:c + 128] = W1[64 * pr["pos0"]:64 * pr["pos0"] + 64]
            w1p[64:128, c:c + 128] = W1[64 * pr["pos1"]:64 * pr["pos1"] + 64]
        for i, e in enumerate(p["singles"]):
            c = W1_COLS[(t, "single", i)]
            half = e["half"]
            w1p[64 * half:64 * half + 64, c:c + 128] = \
                W1[64 * e["pos"]:64 * e["pos"] + 64]
    w2p = np.zeros((128, 128 * 21), np.float32)
    w3p = np.zeros((128, 64 * 21), np.float32)
    b1p = np.zeros((128, 21), np.float32)
    b2p = np.zeros((128, 21), np.float32)
    b3p = np.zeros((128, TPAIRS), np.float32)   # per-partition bias, paired
    for t in range(21):
        n, row, _ = TARGET[t]
        w2p[:, 128 * t:128 * (t + 1)] = np.asarray(inputs[f"w2_g{n}"][row])
        w3p[:, 64 * t:64 * (t + 1)] = np.asarray(inputs[f"w3_g{n}"][row])
        b1p[:, t] = np.asarray(inputs[f"b1_g{n}"][row])
        b2p[:, t] = np.asarray(inputs[f"b2_g{n}"][row])
        b3p[64 * (t % 2):64 * (t % 2) + 64, t // 2] = \
            np.asarray(inputs[f"b3_g{n}"][row])
    return dict(
        w1p=w1p.astype(bf16), w2p=w2p.astype(bf16), w3p=w3p.astype(bf16),
        b1p=b1p, b2p=b2p, b3p=b3p,
    )


def numpy_emulate(inputs, x):
    """Bit-layout-faithful numpy model of the HW kernel (minus PSUM rounding):
    validates the chunk plan / packing / L3 pairing offline."""
    bf16 = ml_dtypes.bfloat16
    packed = pack_weights(inputs)
    xb = x.astype(bf16)  # [Bn, 21, 64]
    Bn = x.shape[0]
    xT = {}
    for p, (u, v) in enumerate(XTILES):
        xT[p] = np.concatenate([xb[:, u], xb[:, v]], 1).T
    out = np.zeros((Bn, 21, 64), np.float32)
    for t in range(21):
        psum1 = np.zeros((128, Bn), np.float32)
        pl = L1_PLAN[t]
        for i, pr in enumerate(pl["pairs"]):
            c = W1_COLS[(t, "pair", i)]
            lhsT = packed["w1p"][:, c:c + 128].astype(np.float32)
            psum1 += lhsT.T @ xT[pr["tile"]].astype(np.float32)
        for i, e in enumerate(pl["singles"]):
            c = W1_COLS[(t, "single", i)]
            lhsT = packed["w1p"][:, c:c + 128].astype(np.float32)
            psum1 += lhsT.T @ xT[e["tile"]].astype(np.float32)
        h1 = np.maximum(psum1 + packed["b1p"][:, t:t + 1], 0).astype(bf16)
        w2 = packed["w2p"][:, 128 * t:128 * (t + 1)].astype(np.float32)
        psum2 = w2.T @ h1.astype(np.float32)
        h2 = np.maximum(psum2 + packed["b2p"][:, t:t + 1], 0).astype(bf16)
        w3 = packed["w3p"][:, 64 * t:64 * (t + 1)].astype(np.float32)
        b3 = packed["b3p"][64 * (t % 2):64 * (t % 2) + 64, t // 2]
        out[:, t] = (w3.T @ h2.astype(np.float32) + b3[:, None]).T
    return out


# ---------------------------------------------------------------------------
# Bass kernel
# ---------------------------------------------------------------------------

# debug knobs: COUPLES=False issues couple singles serially (still K=64);
# L3_PAIR=False gives each target its own psum bank (no col-group pairing).
import os as _os
COUPLES = _os.environ.get("K_COUPLES", "1") == "1"
L3_PAIR = _os.environ.get("K_L3PAIR", "1") == "1"
TMAX = int(_os.environ.get("K_TMAX", "21"))      # process targets [0, TMAX)
L3OFF = _os.environ.get("K_L3OFF", "0") == "1"   # skip L3 (store psum2 raw)
TONLY = ([int(v) for v in _os.environ["K_TONLY"].split(",")]
         if "K_TONLY" in _os.environ else None)


def build_bass_kernel(num_devices=NCORES, bc=BC):
    import concourse.bass as bass
    import concourse.tile as tile
    from concourse import bacc, mybir

    bf16 = mybir.dt.bfloat16
    f32 = mybir.dt.float32
    Relu = mybir.ActivationFunctionType.Relu
    Ident = mybir.ActivationFunctionType.Identity
    Alu = mybir.AluOpType
    ntiles = bc // TILE

    nc = bacc.Bacc("TRN2", target_bir_lowering=False, debug=False,
                   num_devices=num_devices)
    x_dram = nc.dram_tensor("x", [bc, J * D], bf16, kind="ExternalInput").ap()
    out_dram = nc.dram_tensor("outf", [J * D, bc], f32, kind="ExternalOutput").ap()
    w1_dram = nc.dram_tensor("w1p", [128, W1_NCOL], bf16, kind="ExternalInput").ap()
    w2_dram = nc.dram_tensor("w2p", [128, 128 * 21], bf16, kind="ExternalInput").ap()
    w3_dram = nc.dram_tensor("w3p", [128, 64 * 21], bf16, kind="ExternalInput").ap()
    b1_dram = nc.dram_tensor("b1p", [128, 21], f32, kind="ExternalInput").ap()
    b2_dram = nc.dram_tensor("b2p", [128, 21], f32, kind="ExternalInput").ap()
    b3_dram = nc.dram_tensor("b3p", [128, TPAIRS], f32, kind="ExternalInput").ap()

    with tile.TileContext(nc) as tc:
        with (
            tc.tile_pool(name="wpool", bufs=1) as wpool,
            tc.tile_pool(name="xtp", bufs=3) as xtp,
            tc.tile_pool(name="h1p", bufs=2) as h1p,
            tc.tile_pool(name="h2p", bufs=1) as h2p,
            tc.tile_pool(name="outp", bufs=6) as outp,
            tc.tile_pool(name="ps12", bufs=3, space="PSUM") as ps12,
            tc.tile_pool(name="ps3", bufs=2, space="PSUM") as ps3,
        ):
            w1s = wpool.tile([128, W1_NCOL], bf16, name="w1s")
            w2s = wpool.tile([128, 128 * 21], bf16, name="w2s")
            w3s = wpool.tile([128, 64 * 21], bf16, name="w3s")
            b1s = wpool.tile([128, 21], f32, name="b1s")
            b2s = wpool.tile([128, 21], f32, name="b2s")
            b3s = wpool.tile([128, TPAIRS], f32, name="b3s")
            # w1 on the fast scalar HWDGE queue (needed first); the rest on
            # gpsimd so neither HWDGE queue is blocked at startup.
            nc.scalar.dma_start(w1s[:], w1_dram)
            nc.gpsimd.dma_start(w2s[:], w2_dram)
            nc.gpsimd.dma_start(w3s[:], w3_dram)
            nc.gpsimd.dma_start(b1s[:], b1_dram)
            nc.gpsimd.dma_start(b2s[:], b2_dram)
            nc.gpsimd.dma_start(b3s[:], b3_dram)

            # round-robin the L3 evacuation between the two PSUM readers;
            # ScalarE is the faster one and also carries all h1 evacs, so
            # bias the rotation toward VectorE.
            l3_rr = [0]

            # transpose issue order: target-0's tiles first so the first
            # L1 matmuls can start as early as possible
            XORDER = [0, 3, 6, 9, 12, 1, 2, 4, 5, 7, 8, 10, 11, 13, 14]

            def issue_transposes(it):
                """Feature-major pair tiles via DRAM-source xbar transpose.
                All transposes go through ONE queue (sync): the xbar unit is
                a shared resource and concurrent transposes from two queues
                corrupt each other (observed: split sync/scalar issue at it=0
                corrupted two tiles)."""
                b0 = it * TILE
                xT = [None] * NPAIRS
                for p in XORDER:
                    u = XTILES[p][0]
                    xt = xtp.tile([128, TILE], bf16, tag=f"xt{p}", name=f"xt{p}")
                    nc.sync.dma_start(
                        xt[:], x_dram[b0:b0 + TILE, 64 * u:64 * u + 128],
                        transpose=True)
                    xT[p] = xt
                return xT

            # transposes are software-pipelined one iteration ahead so the
            # sync queue issues them BEFORE iter it's store triggers (whose
            # long semaphore waits would otherwise head-of-line-block the
            # next iteration's input tiles).
            # ---- software-pipelined emission ---------------------------
            # PE stream per step k: L1(k) | L2(k-1) | L3(pair done at k-2).
            # The one-stage lag hides the h1/h2 PSUM-evacuation latency
            # (ScalarE/VectorE) behind the next target's L1 matmuls, and the
            # pipeline is continuous across batch-tile boundaries.
            units = [(it, t) for it in range(ntiles) for t in range(21)]
            NU = len(units)
            xts = {0: issue_transposes(0)}
            h1t = {}
            h2t = {}

            def stage_l1(k):
                it, t = units[k]
                if t == 0 and it + 1 < ntiles:
                    xts[it + 1] = issue_transposes(it + 1)
                xT = xts[it]
                pl = L1_PLAN[t]
                psum1 = ps12.tile([128, TILE], f32, tag="ps12", name="ps12")
                chunks = []
                for i, pr in enumerate(pl["pairs"]):
                    chunks.append((W1_COLS[(t, "pair", i)], pr["tile"]))
                for i, e in enumerate(pl["singles"]):
                    chunks.append((W1_COLS[(t, "single", i)], e["tile"]))
                nch = len(chunks)
                for ci, (c, tl) in enumerate(chunks):
                    for h in range(2):
                        nc.tensor.matmul(
                            psum1[:, 512 * h:512 * (h + 1)],
                            w1s[:, c:c + 128],
                            xT[tl][:, 512 * h:512 * (h + 1)],
                            start=(ci == 0), stop=(ci == nch - 1))
                h1 = h1p.tile([128, TILE], bf16, tag="h1", name="h1")
                if t % 2 == 0:
                    nc.scalar.activation(h1[:], psum1[:], Relu,
                                         bias=b1s[:, t:t + 1], scale=1.0)
                else:
                    nc.vector.tensor_scalar(
                        h1[:], psum1[:], b1s[:, t:t + 1], 0.0,
                        Alu.add, Alu.max)
                h1t[k] = h1

            def stage_l2(k):
                it, t = units[k]
                h1 = h1t.pop(k)
                psum2 = ps12.tile([128, TILE], f32, tag="ps12", name="ps12")
                for h in range(2):
                    nc.tensor.matmul(
                        psum2[:, 512 * h:512 * (h + 1)],
                        w2s[:, 128 * t:128 * (t + 1)],
                        h1[:, 512 * h:512 * (h + 1)],
                        start=True, stop=True)
                h2 = h2p.tile([128, TILE], bf16, tag=f"h2_{t % 4}",
                              name=f"h2_{t % 4}")
                if t % 2 == 1:
                    nc.scalar.activation(h2[:], psum2[:], Relu,
                                         bias=b2s[:, t:t + 1], scale=1.0)
                else:
                    nc.vector.tensor_scalar(
                        h2[:], psum2[:], b2s[:, t:t + 1], 0.0,
                        Alu.add, Alu.max)
                h2t[k] = h2

            def stage_l3(k):
                it, t = units[k]
                if not (t % 2 == 1 or t == 20):
                    return
                b0 = it * TILE
                tp = t // 2
                if t % 2 == 1:
                    tlo, thi = t - 1, t
                    h2lo, h2hi = h2t.pop(k - 1), h2t.pop(k)
                    rows = 128
                else:
                    tlo, thi = t, None
                    h2lo, h2hi = h2t.pop(k), None
                    rows = 64
                ot = outp.tile([128, TILE], f32, tag="ot", name="ot")
                ps3a = ps3.tile([128, 512], f32, tag="psum3", name="psum3")
                ps3b = ps3.tile([128, 512], f32, tag="psum3", name="psum3")
                for h in range(2):
                    nc.tensor.matmul(
                        (ps3a if h == 0 else ps3b)[0:64, :],
                        w3s[:, 64 * tlo:64 * tlo + 64],
                        h2lo[:, 512 * h:512 * (h + 1)],
                        start=True, stop=True, skip_group_check=True)
                if thi is not None:
                    for h in range(2):
                        nc.tensor.matmul(
                            (ps3a if h == 0 else ps3b)[64:128, :],
                            w3s[:, 64 * thi:64 * thi + 64],
                            h2hi[:, 512 * h:512 * (h + 1)],
                            start=True, stop=True, skip_group_check=True)
                for h in range(2):
                    psum3 = ps3a if h == 0 else ps3b
                    l3_rr[0] = (l3_rr[0] + 1) % 11
                    if l3_rr[0] < 5:
                        nc.scalar.activation(
                            ot[0:rows, 512 * h:512 * (h + 1)],
                            psum3[0:rows, :], Ident,
                            bias=b3s[0:rows, tp:tp + 1], scale=1.0)
                    else:
                        nc.vector.tensor_scalar(
                            ot[0:rows, 512 * h:512 * (h + 1)],
                            psum3[0:rows, :],
                            b3s[0:rows, tp:tp + 1], None, Alu.add)
                nc.sync.dma_start(
                    out_dram[128 * tp:128 * tp + rows, b0:b0 + TILE],
                    ot[0:rows, :])

            for k in range(NU + 2):
                if k < NU:
                    stage_l1(k)
                if 0 <= k - 1 < NU:
                    stage_l2(k - 1)
                if 0 <= k - 2 < NU:
                    stage_l3(k - 2)

    nc.compile()
    return nc


PACKED = None
_NC = None
LAST_RESULT = None


def prepare(inputs):
    """Build (once) the bass module and the per-core input maps."""
    global PACKED, _NC
    import sys
    if "/opt/trn_rl_repo" not in sys.path:
        sys.path.insert(0, "/opt/trn_rl_repo")
    bf16 = ml_dtypes.bfloat16
    x = np.asarray(inputs["x"], np.float32).reshape(B, J * D).astype(bf16)
    PACKED = pack_weights(inputs)
    if _NC is None:
        _NC = build_bass_kernel()
    in_maps = []
    for core in range(NCORES):
        m = dict(PACKED)
        m["x"] = np.ascontiguousarray(x[core * BC:(core + 1) * BC])
        in_maps.append(m)
    return _NC, in_maps


def kernel(**inputs):
    global LAST_RESULT
    nc, in_maps = prepare(inputs)
    from concourse.bass_utils import run_bass_kernel_spmd
    res = run_bass_kernel_spmd(nc, in_maps, core_ids=list(range(NCORES)))
    LAST_RESULT = res
    # outf is [1344, BC] feature-major per core; unshard + transpose on host.
    out = np.empty((B, J, D), np.float32)
    for core, r in enumerate(res.results):
        fm = r["outf"].reshape(J, D, BC)
        out[core * BC:(core + 1) * BC] = fm.transpose(2, 0, 1)
    return out
